# revision 21
# baseline (speedup 1.0000x reference)
"""Multi-head cross-attention TRN2 kernel (v2).

N=4096, D=256, H=4, K=16. Data-parallel over 8 NeuronCores: each core owns
512 query rows; key_value + weights replicated. No collectives.

Key ideas vs baseline (93.4 us measured -> ~42 us):
- Host pre-transposes q/kv (layout prep only), so no device DMA transposes.
- Projections pack all 4 heads at partition offsets 32h. The S matmuls run
  2-way concurrent via PE row tiling (tile_position=(32h,0)) in head-pair
  instances whose [128,1024] f32 psum tile spans exactly 2 banks, one per
  concurrent row tile -- concurrent row-tiled matmuls into the SAME psum
  bank lock up the device (measured, micro_tile.py). The AV matmuls run
  4-way concurrent via col tiling (tile_position=(0,32h)): same bank is
  fine there because the output partition ranges are disjoint.
- The softmax exp is the wall: 65536 psum->sbuf elems/lane with only
  ScalarE (1/cyc @1.2GHz) and VectorE (1/cyc @0.96GHz, psum port blocks
  the 2x modes) able to read PSUM. It is split across both: ACT does true
  exp; DVE computes bf16 BITS of exp via a Schraudolph step in one
  tensor_scalar: int16(x*128*log2e + 16250.5) bit-viewed as bf16 (~3% per
  element, averages out over 4096 keys; end-to-end rel err 5.5e-3).
  hp0->ACT / hp1->DVE alternation with 3 both-ACT rebalance chunks.
- W_q is pre-scaled by 0.25 (the 1/sqrt(K) softmax scale) on host.
- Denominators ride along as a ones-column in v_aug (row 32h of the AV
  accumulator); normalization is one block-diag-ones broadcast matmul, one
  full-width reciprocal_approx_fast, one tensor_mul.
- Emission order is the execution plan: keep producer->consumer distances
  short. A "smarter" software-pipelined variant (AV delayed one chunk,
  projections dripped between chunks) measured 30x SLOWER end to end.
"""
import numpy as np
import ml_dtypes

import concourse.bass as bass
from concourse import bacc
import concourse.mybir as mybir
import concourse.tile as tile
from concourse.bass_utils import run_bass_kernel_spmd

N, D, H, K = 4096, 256, 4, 16
NCORES = 8
R = N // NCORES          # 512 query rows per core
G = K + 1                # 17: ones column + 16 V dims per head group
NKC = N // 128           # 32 key chunks
F32 = mybir.dt.float32
BF16 = mybir.dt.bfloat16
I16 = mybir.dt.int16
EXPF = mybir.ActivationFunctionType.Exp
MULT = mybir.AluOpType.mult
ADD = mybir.AluOpType.add
BF = ml_dtypes.bfloat16

SCH_MULT = float(128.0 / np.log(2.0))   # 184.664
SCH_BIAS = 16256.0 - 5.5                # Schraudolph magic for bf16 bits

TRACE = False
LAST_RESULTS = None
EXP_PATTERN = "alt"    # "alt": hp0->ACT/hp1->DVE, both-ACT at c in {4,12,20}
                       # (right after proj emission so DVE absorbs proj
                       # copies while exp-free); "burst": idx%15<8 -> ACT


def _build(repeats=1):
    nc = bacc.Bacc()
    qt_d = nc.declare_dram_parameter("qt", [D, R], BF16, isOutput=False)
    kvt_d = nc.declare_dram_parameter("kvt", [D, N], BF16, isOutput=False)
    wq_d = nc.declare_dram_parameter("wq", [D, 128], BF16, isOutput=False)
    wk_d = nc.declare_dram_parameter("wk", [D, 128], BF16, isOutput=False)
    wv_d = nc.declare_dram_parameter("wv", [D, 68], BF16, isOutput=False)
    wo_d = nc.declare_dram_parameter("wo", [128, D], BF16, isOutput=False)
    on4_d = nc.declare_dram_parameter("on4", [128, 128], BF16, isOutput=False)
    out_d = nc.declare_dram_parameter("out", [R, D], F32, isOutput=True)

    with tile.TileContext(nc) as tc:
        with (
            tc.tile_pool(name="consts", bufs=1) as consts,
            tc.tile_pool(name="es", bufs=3) as espool,
            tc.tile_pool(name="sbops", bufs=2) as sbops,
            tc.tile_pool(name="sp", bufs=2, space="PSUM") as spool,
            tc.tile_pool(name="avp", bufs=1, space="PSUM") as avpool,
            tc.tile_pool(name="mp", bufs=1, space="PSUM") as mpool,
        ):
            for _rep in range(repeats):
                # ---------- DMA in ----------
                wq_sb = consts.tile([128, 256], BF16, tag="wq", name="wq")
                wk_sb = consts.tile([128, 256], BF16, tag="wk", name="wk")
                wv_sb = consts.tile([128, 136], BF16, tag="wv", name="wv")
                wo_sb = consts.tile([128, 256], BF16, tag="wo", name="wo")
                on4_sb = consts.tile([128, 128], BF16, tag="on4", name="on4")
                for half in range(2):
                    dsl = slice(128 * half, 128 * (half + 1))
                    nc.sync.dma_start(out=wq_sb[:, 128 * half:128 * half + 128],
                                      in_=wq_d[dsl, :])
                    nc.sync.dma_start(out=wk_sb[:, 128 * half:128 * half + 128],
                                      in_=wk_d[dsl, :])
                    nc.sync.dma_start(out=wv_sb[:, 68 * half:68 * half + 68],
                                      in_=wv_d[dsl, :])
                nc.sync.dma_start(out=wo_sb, in_=wo_d[:, :])
                nc.sync.dma_start(out=on4_sb, in_=on4_d[:, :])

                qt_raw = consts.tile([128, 1024], BF16, tag="qtr", name="qtr")
                nc.sync.dma_start(out=qt_raw[:, 0:512], in_=qt_d[0:128, :])
                nc.sync.dma_start(out=qt_raw[:, 512:1024], in_=qt_d[128:256, :])

                kt = consts.tile([128, 8192], BF16, tag="kt", name="kt")
                for j in range(8):
                    tsl = slice(512 * j, 512 * (j + 1))
                    nc.sync.dma_start(out=kt[:, 512 * j:512 * (j + 1)],
                                      in_=kvt_d[0:128, tsl])
                    nc.sync.dma_start(out=kt[:, 4096 + 512 * j:4096 + 512 * (j + 1)],
                                      in_=kvt_d[128:256, tsl])

                # ---------- persistent SBUF results ----------
                kht = consts.tile([128, N], BF16, tag="kht", name="kht")
                v_aug = consts.tile([128, NKC * 68], BF16, tag="v_aug", name="v_aug")
                qt_sb = consts.tile([128, 512], BF16, tag="qt_sb", name="qt_sb")
                hn_sb = consts.tile([128, 512], BF16, tag="hn_sb", name="hn_sb")

                # ones slots of v_aug (col 68c + 17h), written once on gpsimd
                von = v_aug[:].rearrange("p (i h s) -> p i h s", i=NKC, h=H, s=G)[:, :, :, 0:1]
                nc.gpsimd.memset(von, 1.0)

                # AV accumulator: zero data so never-written rows stay finite
                av_ps = avpool.tile([128, 512], F32, tag="av", name="av")
                nc.vector.memset(av_ps[:], 0.0)

                # ---------- QT projection ----------
                qt_psum = mpool.tile([128, 512], F32, tag="kh", name="kh")
                nc.tensor.matmul(qt_psum[:], wq_sb[:, 0:128], qt_raw[:, 0:512],
                                 start=True, stop=False)
                nc.tensor.matmul(qt_psum[:], wq_sb[:, 128:256], qt_raw[:, 512:1024],
                                 start=False, stop=True)
                nc.scalar.copy(qt_sb[:], qt_psum[:])

                # ---------- interleaved projections + attention ----------
                def proj(j):
                    kh_psum = mpool.tile([128, 512], F32, tag="kh", name="kh")
                    tsl = slice(512 * j, 512 * (j + 1))
                    nc.tensor.matmul(kh_psum[:], wk_sb[:, 0:128], kt[:, tsl],
                                     start=True, stop=False)
                    nc.tensor.matmul(kh_psum[:], wk_sb[:, 128:256],
                                     kt[:, 4096 + 512 * j:4096 + 512 * (j + 1)],
                                     start=False, stop=True)
                    if j % 2 == 0:
                        nc.scalar.copy(kht[:, tsl], kh_psum[:])
                    else:
                        nc.vector.tensor_copy(kht[:, tsl], kh_psum[:])

                    v_psum = mpool.tile([128, 272], F32, tag="v", name="v")
                    for s in range(4):
                        i = 4 * j + s
                        nc.tensor.matmul(v_psum[:, 68 * s:68 * (s + 1)],
                                         kt[:, 128 * i:128 * (i + 1)],
                                         wv_sb[:, 0:68], start=True, stop=False)
                        nc.tensor.matmul(v_psum[:, 68 * s:68 * (s + 1)],
                                         kt[:, 4096 + 128 * i:4096 + 128 * (i + 1)],
                                         wv_sb[:, 68:136], start=False, stop=True)
                    # copy the 16 V cols of each head group (skip ones col)
                    vsrc = v_psum[:].rearrange("p (s h g) -> p s h g", s=4, g=G)[:, :, :, 1:G]
                    vdst = v_aug[:, 272 * j:272 * (j + 1)].rearrange(
                        "p (s h g) -> p s h g", s=4, g=G)[:, :, :, 1:G]
                    if j % 2 == 0:
                        nc.vector.tensor_copy(vdst, vsrc)
                    else:
                        nc.scalar.copy(vdst, vsrc)

                first_av = [True]

                def attn(c):
                    # S: two head-pair instances; each [128,1024] f32 tile is
                    # exactly 2 PSUM banks, one per concurrent row-tile
                    # (same-bank concurrent row tiles are fatal on TRN2 HW).
                    es_tiles = []
                    for hp in range(2):
                        s_ps = spool.tile([128, 1024], F32, tag="s", name="s")
                        for i in range(2):
                            h = 2 * hp + i
                            nc.tensor.matmul(
                                s_ps[:, 512 * i:512 * (i + 1)],
                                kht[32 * h:32 * h + 16, 128 * c:128 * (c + 1)],
                                qt_sb[32 * h:32 * h + 16, :],
                                start=True, stop=True, tile_position=(32 * h, 0))
                        es = espool.tile([128, 1024], BF16, tag="es", name="es")
                        # hp=0 -> ACT (gates next chunk's S via slot rotation,
                        # and ACT is the faster engine); hp=1 -> DVE; a few
                        # both-ACT chunks rebalance total load (ACT 35 : DVE 29)
                        if EXP_PATTERN == "alt":
                            use_act = hp == 0 or c in (4, 12, 20)
                        else:
                            use_act = (2 * c + hp) % 15 < 8
                        if use_act:
                            nc.scalar.activation(es[:], s_ps[:], EXPF, scale=1.0)
                        else:
                            nc.vector.tensor_scalar(
                                es[:].bitcast(I16), s_ps[:], SCH_MULT, SCH_BIAS,
                                MULT, ADD)
                        es_tiles.append(es)
                    for h in range(H):
                        nc.tensor.matmul(
                            av_ps[32 * h:32 * h + G, :],
                            v_aug[:, 68 * c + 17 * h:68 * c + 17 * h + G],
                            es_tiles[h // 2][:, 512 * (h % 2):512 * (h % 2 + 1)],
                            start=first_av[0], stop=(c == NKC - 1 and h == H - 1),
                            tile_position=(0, 32 * h), skip_group_check=True)
                        first_av[0] = False

                proj(0)
                proj(1)
                for j in range(2, 10):
                    for c in range(4 * (j - 2), 4 * (j - 1)):
                        attn(c)
                    if j < 8:
                        proj(j)

                # ---------- normalize + W_o + out ----------
                # broadcast raw denominators (av_ps row 32h) to every row of
                # head h's 32-block: rb[p, q] = den[p // 32, q], then one
                # full-width approx reciprocal and one multiply.
                av_sb = sbops.tile([128, 512], BF16, tag="av_sb", name="av_sb")
                nc.scalar.copy(av_sb[:], av_ps[:])
                rb_ps = spool.tile([128, 1024], F32, tag="s", name="s")
                nc.tensor.matmul(rb_ps[:, 0:512], on4_sb[:], av_sb[:],
                                 start=True, stop=True)
                recip_sb = sbops.tile([128, 512], F32, tag="recip_sb",
                                      name="recip_sb")
                nc.vector.reciprocal_approx_fast(recip_sb[:], rb_ps[:, 0:512])
                nc.vector.tensor_mul(hn_sb[:], av_ps[:], recip_sb[:])

                for half in range(2):
                    wo_ps = mpool.tile([128, 512], F32, tag="kh", name="kh")
                    for tt in range(2):
                        t = 2 * half + tt
                        nc.tensor.matmul(wo_ps[:, 256 * tt:256 * (tt + 1)],
                                         hn_sb[:, 128 * t:128 * (t + 1)],
                                         wo_sb[:], start=True, stop=True)
                    osb = sbops.tile([128, 512], F32, tag="osb", name="osb")
                    if half == 0:
                        nc.scalar.copy(osb[:], wo_ps[:])
                    else:
                        nc.vector.tensor_copy(osb[:], wo_ps[:])
                    odst = out_d[256 * half:256 * (half + 1), :].rearrange(
                        "(tt p) d -> p tt d", tt=2)
                    nc.sync.dma_start(out=odst, in_=osb[:].rearrange(
                        "p (tt d) -> p tt d", tt=2))

    nc.finalize()
    return nc


_NC_CACHE = None


def _host_in_maps(query, key_value, W_q, W_k, W_v, W_o):
    qt = np.ascontiguousarray(query.astype(BF).T)        # [D, N]
    kvt = np.ascontiguousarray(key_value.astype(BF).T)   # [D, N]
    wq = np.zeros((D, 128), dtype=BF)
    wk = np.zeros((D, 128), dtype=BF)
    wv = np.zeros((D, 68), dtype=BF)
    for h in range(H):
        wq[:, 32 * h:32 * h + K] = (W_q[h] * 0.25).astype(BF)
        wk[:, 32 * h:32 * h + K] = W_k[h].astype(BF)
        wv[:, 17 * h + 1:17 * (h + 1)] = W_v[h].astype(BF)
    wo = np.zeros((128, D), dtype=BF)
    wo_r = W_o.reshape(H, K, D)
    for h in range(H):
        wo[32 * h + 1:32 * h + 1 + K, :] = wo_r[h].astype(BF)
    on4 = np.zeros((128, 128), dtype=BF)
    for g in range(4):
        on4[32 * g, 32 * g:32 * (g + 1)] = 1.0
    return [{"qt": np.ascontiguousarray(qt[:, c * R:(c + 1) * R]), "kvt": kvt,
             "wq": wq, "wk": wk, "wv": wv, "wo": wo, "on4": on4}
            for c in range(NCORES)]


def kernel(query, key_value, W_q, W_k, W_v, W_o):
    global _NC_CACHE, LAST_RESULTS
    if _NC_CACHE is None:
        _NC_CACHE = _build()
    nc = _NC_CACHE
    in_maps = _host_in_maps(query, key_value, W_q, W_k, W_v, W_o)
    res = run_bass_kernel_spmd(nc, in_maps, list(range(NCORES)), trace=TRACE)
    LAST_RESULTS = res
    return np.concatenate([res.results[c]["out"] for c in range(NCORES)], axis=0)


# revision 28
# speedup vs baseline: 170.9690x; 170.9690x over previous
"""Multi-head cross-attention TRN2 kernel (v2).

N=4096, D=256, H=4, K=16. Data-parallel over 8 NeuronCores: each core owns
512 query rows; key_value + weights replicated. No collectives.

Key ideas vs baseline (93.4 us measured -> ~42 us):
- Host pre-transposes q/kv (layout prep only), so no device DMA transposes.
- Projections pack all 4 heads at partition offsets 32h. The S matmuls run
  2-way concurrent via PE row tiling (tile_position=(32h,0)) in head-pair
  instances whose [128,1024] f32 psum tile spans exactly 2 banks, one per
  concurrent row tile -- concurrent row-tiled matmuls into the SAME psum
  bank lock up the device (measured, micro_tile.py). The AV matmuls run
  4-way concurrent via col tiling (tile_position=(0,32h)): same bank is
  fine there because the output partition ranges are disjoint.
- The softmax exp is the wall: 65536 psum->sbuf elems/lane with only
  ScalarE (1/cyc @1.2GHz) and VectorE (1/cyc @0.96GHz, psum port blocks
  the 2x modes) able to read PSUM. It is split across both: ACT does true
  exp; DVE computes bf16 BITS of exp via a Schraudolph step in one
  tensor_scalar: int16(x*128*log2e + 16250.5) bit-viewed as bf16 (~3% per
  element, averages out over 4096 keys; end-to-end rel err 5.5e-3).
  hp0->ACT / hp1->DVE alternation with 3 both-ACT rebalance chunks.
- W_q is pre-scaled by 0.25 (the 1/sqrt(K) softmax scale) on host.
- Denominators ride along as a ones-column in v_aug (row 32h of the AV
  accumulator); normalization is one block-diag-ones broadcast matmul, one
  full-width reciprocal_approx_fast, one tensor_mul.
- KhT projection psum is [128,1024] over two token-chunks (2 banks, uses
  the spare 8th PSUM bank) so kht needs 4 big copies instead of 8; V
  copies all on DVE; exp split rebalanced 36 ACT / 28 DVE to equalize
  engine busy (~41.3us each by the cost model).
- Emission order is the execution plan: keep producer->consumer distances
  short. A "smarter" software-pipelined variant (AV delayed one chunk,
  projections dripped between chunks) measured 30x SLOWER end to end.
"""
import numpy as np
import ml_dtypes

import concourse.bass as bass
from concourse import bacc
import concourse.mybir as mybir
import concourse.tile as tile
from concourse.bass_utils import run_bass_kernel_spmd

N, D, H, K = 4096, 256, 4, 16
NCORES = 8
R = N // NCORES          # 512 query rows per core
G = K + 1                # 17: ones column + 16 V dims per head group
NKC = N // 128           # 32 key chunks
F32 = mybir.dt.float32
BF16 = mybir.dt.bfloat16
I16 = mybir.dt.int16
EXPF = mybir.ActivationFunctionType.Exp
MULT = mybir.AluOpType.mult
ADD = mybir.AluOpType.add
BF = ml_dtypes.bfloat16

SCH_MULT = float(128.0 / np.log(2.0))   # 184.664
SCH_BIAS = 16256.0 - 5.5                # Schraudolph magic for bf16 bits

TRACE = False
LAST_RESULTS = None
EXP_PATTERN = "alt"    # "alt": hp0->ACT/hp1->DVE, both-ACT at c in {4,12,20}
                       # (right after proj emission so DVE absorbs proj
                       # copies while exp-free); "burst": idx%15<8 -> ACT


def _build(repeats=1):
    nc = bacc.Bacc()
    qt_d = nc.declare_dram_parameter("qt", [D, R], BF16, isOutput=False)
    kvt_d = nc.declare_dram_parameter("kvt", [D, N], BF16, isOutput=False)
    wq_d = nc.declare_dram_parameter("wq", [D, 128], BF16, isOutput=False)
    wk_d = nc.declare_dram_parameter("wk", [D, 128], BF16, isOutput=False)
    wv_d = nc.declare_dram_parameter("wv", [D, 68], BF16, isOutput=False)
    wo_d = nc.declare_dram_parameter("wo", [128, D], BF16, isOutput=False)
    on4_d = nc.declare_dram_parameter("on4", [128, 128], BF16, isOutput=False)
    out_d = nc.declare_dram_parameter("out", [R, D], F32, isOutput=True)

    with tile.TileContext(nc) as tc:
        with (
            tc.tile_pool(name="consts", bufs=1) as consts,
            tc.tile_pool(name="es", bufs=3) as espool,
            tc.tile_pool(name="sbops", bufs=2) as sbops,
            tc.tile_pool(name="sp", bufs=2, space="PSUM") as spool,
            tc.tile_pool(name="avp", bufs=1, space="PSUM") as avpool,
            tc.tile_pool(name="mp", bufs=1, space="PSUM") as mpool,
        ):
            for _rep in range(repeats):
                # ---------- DMA in ----------
                wq_sb = consts.tile([128, 256], BF16, tag="wq", name="wq")
                wk_sb = consts.tile([128, 256], BF16, tag="wk", name="wk")
                wv_sb = consts.tile([128, 136], BF16, tag="wv", name="wv")
                wo_sb = consts.tile([128, 256], BF16, tag="wo", name="wo")
                on4_sb = consts.tile([128, 128], BF16, tag="on4", name="on4")
                for half in range(2):
                    dsl = slice(128 * half, 128 * (half + 1))
                    nc.sync.dma_start(out=wq_sb[:, 128 * half:128 * half + 128],
                                      in_=wq_d[dsl, :])
                    nc.sync.dma_start(out=wk_sb[:, 128 * half:128 * half + 128],
                                      in_=wk_d[dsl, :])
                    nc.sync.dma_start(out=wv_sb[:, 68 * half:68 * half + 68],
                                      in_=wv_d[dsl, :])
                nc.sync.dma_start(out=wo_sb, in_=wo_d[:, :])
                nc.sync.dma_start(out=on4_sb, in_=on4_d[:, :])

                qt_raw = consts.tile([128, 1024], BF16, tag="qtr", name="qtr")
                nc.sync.dma_start(out=qt_raw[:, 0:512], in_=qt_d[0:128, :])
                nc.sync.dma_start(out=qt_raw[:, 512:1024], in_=qt_d[128:256, :])

                kt = consts.tile([128, 8192], BF16, tag="kt", name="kt")
                for j in range(8):
                    tsl = slice(512 * j, 512 * (j + 1))
                    nc.sync.dma_start(out=kt[:, 512 * j:512 * (j + 1)],
                                      in_=kvt_d[0:128, tsl])
                    nc.sync.dma_start(out=kt[:, 4096 + 512 * j:4096 + 512 * (j + 1)],
                                      in_=kvt_d[128:256, tsl])

                # ---------- persistent SBUF results ----------
                kht = consts.tile([128, N], BF16, tag="kht", name="kht")
                v_aug = consts.tile([128, NKC * 68], BF16, tag="v_aug", name="v_aug")
                qt_sb = consts.tile([128, 512], BF16, tag="qt_sb", name="qt_sb")
                hn_sb = consts.tile([128, 512], BF16, tag="hn_sb", name="hn_sb")

                # ones slots of v_aug (col 68c + 17h), written once on gpsimd
                von = v_aug[:].rearrange("p (i h s) -> p i h s", i=NKC, h=H, s=G)[:, :, :, 0:1]
                nc.gpsimd.memset(von, 1.0)

                # AV accumulator: zero data so never-written rows stay finite
                av_ps = avpool.tile([128, 512], F32, tag="av", name="av")
                nc.vector.memset(av_ps[:], 0.0)

                # ---------- QT projection ----------
                qt_psum = mpool.tile([128, 1024], F32, tag="kh", name="kh")
                nc.tensor.matmul(qt_psum[:, 0:512], wq_sb[:, 0:128], qt_raw[:, 0:512],
                                 start=True, stop=False)
                nc.tensor.matmul(qt_psum[:, 0:512], wq_sb[:, 128:256], qt_raw[:, 512:1024],
                                 start=False, stop=True)
                nc.scalar.copy(qt_sb[:], qt_psum[:, 0:512])

                # ---------- interleaved projections + attention ----------
                # kh psum [128,1024] spans token-chunks 2jp,2jp+1 (2 banks);
                # emission rhythm is unchanged: proj_a emits chunk 2jp's
                # matmuls, proj_b emits chunk 2jp+1's plus ONE combined copy.
                kh_store = {}

                def v_piece(j):
                    v_psum = mpool.tile([128, 272], F32, tag="v", name="v")
                    for s in range(4):
                        i = 4 * j + s
                        nc.tensor.matmul(v_psum[:, 68 * s:68 * (s + 1)],
                                         kt[:, 128 * i:128 * (i + 1)],
                                         wv_sb[:, 0:68], start=True, stop=False)
                        nc.tensor.matmul(v_psum[:, 68 * s:68 * (s + 1)],
                                         kt[:, 4096 + 128 * i:4096 + 128 * (i + 1)],
                                         wv_sb[:, 68:136], start=False, stop=True)
                    # copy the 16 V cols of each head group (skip ones col)
                    vsrc = v_psum[:].rearrange("p (s h g) -> p s h g", s=4, g=G)[:, :, :, 1:G]
                    vdst = v_aug[:, 272 * j:272 * (j + 1)].rearrange(
                        "p (s h g) -> p s h g", s=4, g=G)[:, :, :, 1:G]
                    nc.vector.tensor_copy(vdst, vsrc)

                def kh_mms(kh_psum, j, jj):
                    nc.tensor.matmul(kh_psum[:, 512 * jj:512 * (jj + 1)],
                                     wk_sb[:, 0:128], kt[:, 512 * j:512 * (j + 1)],
                                     start=True, stop=False)
                    nc.tensor.matmul(kh_psum[:, 512 * jj:512 * (jj + 1)],
                                     wk_sb[:, 128:256],
                                     kt[:, 4096 + 512 * j:4096 + 512 * (j + 1)],
                                     start=False, stop=True)

                def proj(j):
                    jp, jj = j // 2, j % 2
                    if jj == 0:
                        kh_store[jp] = mpool.tile([128, 1024], F32, tag="kh",
                                                  name="kh")
                        kh_mms(kh_store[jp], j, 0)
                    else:
                        kh_psum = kh_store.pop(jp)
                        kh_mms(kh_psum, j, 1)
                        ksl = slice(1024 * jp, 1024 * (jp + 1))
                        if jp % 2 == 0:
                            nc.scalar.copy(kht[:, ksl], kh_psum[:])
                        else:
                            nc.vector.tensor_copy(kht[:, ksl], kh_psum[:])
                    v_piece(j)

                first_av = [True]

                def attn(c):
                    # S: two head-pair instances; each [128,1024] f32 tile is
                    # exactly 2 PSUM banks, one per concurrent row-tile
                    # (same-bank concurrent row tiles are fatal on TRN2 HW).
                    es_tiles = []
                    for hp in range(2):
                        s_ps = spool.tile([128, 1024], F32, tag="s", name="s")
                        for i in range(2):
                            h = 2 * hp + i
                            nc.tensor.matmul(
                                s_ps[:, 512 * i:512 * (i + 1)],
                                kht[32 * h:32 * h + 16, 128 * c:128 * (c + 1)],
                                qt_sb[32 * h:32 * h + 16, :],
                                start=True, stop=True, tile_position=(32 * h, 0))
                        es = espool.tile([128, 1024], BF16, tag="es", name="es")
                        # hp=0 -> ACT (gates next chunk's S via slot rotation,
                        # and ACT is the faster engine); hp=1 -> DVE; a few
                        # both-ACT chunks rebalance total load (ACT 35 : DVE 29)
                        if EXP_PATTERN == "alt":
                            use_act = hp == 0 or c in (4, 12, 20, 28)
                        else:
                            use_act = (2 * c + hp) % 15 < 8
                        if use_act:
                            nc.scalar.activation(es[:], s_ps[:], EXPF, scale=1.0)
                        else:
                            nc.vector.tensor_scalar(
                                es[:].bitcast(I16), s_ps[:], SCH_MULT, SCH_BIAS,
                                MULT, ADD)
                        es_tiles.append(es)
                    for h in range(H):
                        nc.tensor.matmul(
                            av_ps[32 * h:32 * h + G, :],
                            v_aug[:, 68 * c + 17 * h:68 * c + 17 * h + G],
                            es_tiles[h // 2][:, 512 * (h % 2):512 * (h % 2 + 1)],
                            start=first_av[0], stop=(c == NKC - 1 and h == H - 1),
                            tile_position=(0, 32 * h), skip_group_check=True)
                        first_av[0] = False

                proj(0)
                proj(1)
                for j in range(2, 10):
                    for c in range(4 * (j - 2), 4 * (j - 1)):
                        attn(c)
                    if j < 8:
                        proj(j)

                # ---------- normalize + W_o + out ----------
                # broadcast raw denominators (av_ps row 32h) to every row of
                # head h's 32-block: rb[p, q] = den[p // 32, q], then approx
                # reciprocal + multiply + W_o + copy-out, pipelined in two
                # query-halves so half 0's W_o/copy/DMA overlaps half 1's
                # reciprocal/normalize.
                av_sb = sbops.tile([128, 512], BF16, tag="av_sb", name="av_sb")
                nc.scalar.copy(av_sb[:], av_ps[:])
                rb_ps = spool.tile([128, 1024], F32, tag="s", name="s")
                nc.tensor.matmul(rb_ps[:, 0:512], on4_sb[:], av_sb[:],
                                 start=True, stop=True)
                recip_sb = sbops.tile([128, 512], F32, tag="recip_sb",
                                      name="recip_sb")
                for half in range(2):
                    qsl = slice(256 * half, 256 * (half + 1))
                    nc.vector.reciprocal_approx_fast(recip_sb[:, qsl],
                                                     rb_ps[:, qsl])
                    nc.vector.tensor_mul(hn_sb[:, qsl], av_ps[:, qsl],
                                         recip_sb[:, qsl])
                    wo_ps = mpool.tile([128, 1024], F32, tag="kh", name="kh")
                    for tt in range(2):
                        t = 2 * half + tt
                        nc.tensor.matmul(wo_ps[:, 256 * tt:256 * (tt + 1)],
                                         hn_sb[:, 128 * t:128 * (t + 1)],
                                         wo_sb[:], start=True, stop=True)
                    osb = sbops.tile([128, 512], F32, tag="osb", name="osb")
                    nc.scalar.copy(osb[:], wo_ps[:, 0:512])
                    odst = out_d[256 * half:256 * (half + 1), :].rearrange(
                        "(tt p) d -> p tt d", tt=2)
                    nc.sync.dma_start(out=odst, in_=osb[:].rearrange(
                        "p (tt d) -> p tt d", tt=2))

    nc.finalize()
    return nc


_NC_CACHE = None


def _host_in_maps(query, key_value, W_q, W_k, W_v, W_o):
    qt = np.ascontiguousarray(query.astype(BF).T)        # [D, N]
    kvt = np.ascontiguousarray(key_value.astype(BF).T)   # [D, N]
    wq = np.zeros((D, 128), dtype=BF)
    wk = np.zeros((D, 128), dtype=BF)
    wv = np.zeros((D, 68), dtype=BF)
    for h in range(H):
        wq[:, 32 * h:32 * h + K] = (W_q[h] * 0.25).astype(BF)
        wk[:, 32 * h:32 * h + K] = W_k[h].astype(BF)
        wv[:, 17 * h + 1:17 * (h + 1)] = W_v[h].astype(BF)
    wo = np.zeros((128, D), dtype=BF)
    wo_r = W_o.reshape(H, K, D)
    for h in range(H):
        wo[32 * h + 1:32 * h + 1 + K, :] = wo_r[h].astype(BF)
    on4 = np.zeros((128, 128), dtype=BF)
    for g in range(4):
        on4[32 * g, 32 * g:32 * (g + 1)] = 1.0
    return [{"qt": np.ascontiguousarray(qt[:, c * R:(c + 1) * R]), "kvt": kvt,
             "wq": wq, "wk": wk, "wv": wv, "wo": wo, "on4": on4}
            for c in range(NCORES)]


def kernel(query, key_value, W_q, W_k, W_v, W_o):
    global _NC_CACHE, LAST_RESULTS
    if _NC_CACHE is None:
        _NC_CACHE = _build()
    nc = _NC_CACHE
    in_maps = _host_in_maps(query, key_value, W_q, W_k, W_v, W_o)
    res = run_bass_kernel_spmd(nc, in_maps, list(range(NCORES)), trace=TRACE)
    LAST_RESULTS = res
    return np.concatenate([res.results[c]["out"] for c in range(NCORES)], axis=0)
